# revision 8
# baseline (speedup 1.0000x reference)
"""GroupedQueryAttention TRN2 Bass kernel, 8-way tensor-parallel over heads.

B=2, S=2048, E=2048, H=16 q-heads, KVH=4 kv-heads, HD=128.
Core d handles q-heads {2d, 2d+1} and kv-head d//2 (redundantly with its pair).

Layout strategy (everything transposed so all matmuls use natural layouts):
  phase 1: qT/kT/vT[hd, tok] = W.T @ xT  (lhsT = W chunks, rhs = xT chunks)
           + RoPE applied on PSUM->SBUF epilogue (1/sqrt(HD) folded into wq)
  phase 1.5: v = transpose(vT) via PE transpose (needed as ctx-matmul lhsT)
  attention (per batch, head, 512-wide q tile): flash-style over PAIRS of
           128-wide kt chunks: scoresT[kt, qt] = kT_chunk.T @ qT_tile -> one
           exp over both chunks [128,1024] (no max subtraction; scores are
           ~N(0,1)) -> causal mask on diagonal chunks -> ctxT[hd, qt] +=
           v_chunk.T @ P; sumexp[1, qt] += ones.T @ P.  PE runs one chunk
           pair ahead of ACT so exp latency is hidden.  Normalize ctxT with
           broadcast (K=1 ones matmul) + reciprocal_approx_fast.
  out_proj (per batch, right after that batch's attention so its output DMA
           overlaps the next batch's attention): out[tok, e] = sum_h
           ctxT_h_chunk.T @ wo_h (partial over this core's 256 head dims;
           host sums the 8 partials).

All matmul operands are float32r (full PE rate at N>=512; ~1e-3 precision).
"""
import sys
sys.path.insert(0, '/opt/trn_rl_repo')

import numpy as np
from contextlib import ExitStack

import concourse.bass as bass
import concourse.bacc as bacc
import concourse.tile as tile
from concourse import mybir
from concourse.bass_utils import run_bass_kernel_spmd
from concourse.alu_op_type import AluOpType

F32 = mybir.dt.float32
F32R = mybir.dt.float32r
EXP = mybir.ActivationFunctionType.Exp

B, S, E = 2, 2048, 2048
H, KVH, HD = 16, 4, 128
T = B * S                  # 4096 flat tokens
NCORES = 8
NT = 512                   # token tile (matmul free dim)
NTT = T // NT              # 8 token tiles
KC = E // 128              # 16 contraction chunks for projections
QT_PER_B = S // NT         # 4 q-tiles per batch
KB_PER_B = S // 128        # 16 kt chunks per batch
ROPE_BASE = 10000.0

_CACHE = {}


def _emit(nc, tc, ctx):
    xT_d = nc.declare_dram_parameter("xT", [E, T], F32R, isOutput=False)
    wq_d = nc.declare_dram_parameter("wq", [E, 2 * HD], F32R, isOutput=False)
    wk_d = nc.declare_dram_parameter("wk", [E, HD], F32R, isOutput=False)
    wv_d = nc.declare_dram_parameter("wv", [E, HD], F32R, isOutput=False)
    wo_d = nc.declare_dram_parameter("wo", [2 * HD, E], F32R, isOutput=False)
    cos_d = nc.declare_dram_parameter("cos", [HD, T], F32, isOutput=False)
    sinm_d = nc.declare_dram_parameter("sinm", [HD, T], F32, isOutput=False)
    masks_d = nc.declare_dram_parameter("masks", [4, 128, NT], F32, isOutput=False)
    ident_d = nc.declare_dram_parameter("ident", [128, 128], F32R, isOutput=False)
    onec_d = nc.declare_dram_parameter("onec", [128, 1], F32R, isOutput=False)
    oner_d = nc.declare_dram_parameter("oner", [1, 128], F32R, isOutput=False)
    out_d = nc.declare_dram_parameter("out", [T, E], F32, isOutput=True)

    persist = ctx.enter_context(tc.tile_pool(name="persist", bufs=1))
    qT0 = persist.tile([HD, T], F32R)
    qT1 = persist.tile([HD, T], F32R)
    kT = persist.tile([HD, T], F32R)
    v_sb = persist.tile([128, T // 128, HD], F32R)   # v natural: [tok%128, blk, hd]
    ctx0 = persist.tile([HD, T], F32R)
    ctx1 = persist.tile([HD, T], F32R)
    ident = persist.tile([128, 128], F32R)
    ones_col = persist.tile([128, 1], F32R)
    ones_row = persist.tile([1, 128], F32R)
    nc.sync.dma_start(ident[:], ident_d[:, :])
    nc.sync.dma_start(ones_col[:], onec_d[:, :])
    nc.sync.dma_start(ones_row[:], oner_d[:, :])

    # ---------------- phase 1: projections + RoPE ----------------
    with ExitStack() as p1x:
        vTp = p1x.enter_context(tc.tile_pool(name="vTp", bufs=1))
        vT = vTp.tile([HD, T], F32R)
        with ExitStack() as p1:
            wpool = p1.enter_context(tc.tile_pool(name="wpool", bufs=1))
            trig = p1.enter_context(tc.tile_pool(name="trig", bufs=1))
            xpool = p1.enter_context(tc.tile_pool(name="xpool", bufs=3))
            rope = p1.enter_context(tc.tile_pool(name="rope", bufs=2))
            ps1 = p1.enter_context(tc.tile_pool(name="ps1", bufs=2, space="PSUM"))

            wq_s = wpool.tile([128, KC, 2 * HD], F32R)
            wk_s = wpool.tile([128, KC, HD], F32R)
            wv_s = wpool.tile([128, KC, HD], F32R)
            cos_s = trig.tile([HD, T], F32)
            sinm_s = trig.tile([HD, T], F32)
            # weight chunk-0 + first x tile first, so the first matmuls
            # start ~15us in instead of waiting for all 12MB of constants
            wqv = wq_d.rearrange("(k p) m -> p k m", p=128)
            wkv = wk_d.rearrange("(k p) m -> p k m", p=128)
            wvv = wv_d.rearrange("(k p) m -> p k m", p=128)
            xT_view = xT_d.rearrange("(k p) t -> p k t", p=128)

            def load_wchunk(kq):
                ks = slice(4 * kq, 4 * kq + 4)
                nc.sync.dma_start(wq_s[:, ks, :], wqv[:, ks, :])
                nc.sync.dma_start(wk_s[:, ks, :], wkv[:, ks, :])
                nc.sync.dma_start(wv_s[:, ks, :], wvv[:, ks, :])

            def load_xk(tt):
                t0 = tt * NT
                xk = []
                for kq in range(4):  # 4 DMAs x 4 chunks of [128, NT]
                    xt = xpool.tile([128, 4, NT], F32R, tag="xk")
                    nc.sync.dma_start(
                        xt[:], xT_view[:, 4 * kq:4 * kq + 4, t0:t0 + NT])
                    xk.append(xt)
                return xk

            load_wchunk(0)
            xk_next = load_xk(0)
            for kq in range(1, 4):
                load_wchunk(kq)
            nc.sync.dma_start(cos_s[:], cos_d[:, :])
            nc.sync.dma_start(sinm_s[:], sinm_d[:, :])

            for tt in range(NTT):
                t0 = tt * NT
                xk = xk_next
                if tt + 1 < NTT:
                    xk_next = load_xk(tt + 1)
                pq0 = ps1.tile([HD, NT], F32, tag="pq0")
                pq1 = ps1.tile([HD, NT], F32, tag="pq1")
                pk = ps1.tile([HD, NT], F32, tag="pk")
                pv = ps1.tile([HD, NT], F32, tag="pv")
                for k in range(KC):
                    xck = xk[k // 4][:, k % 4, :]
                    st, sp = (k == 0), (k == KC - 1)
                    nc.tensor.matmul(pq0[:], wq_s[:, k, 0:HD], xck, start=st, stop=sp)
                    nc.tensor.matmul(pq1[:], wq_s[:, k, HD:2 * HD], xck, start=st, stop=sp)
                    nc.tensor.matmul(pk[:], wk_s[:, k, :], xck, start=st, stop=sp)
                    nc.tensor.matmul(pv[:], wv_s[:, k, :], xck, start=st, stop=sp)
                # RoPE: q' = q*cos + swap_halves(q)*sinm  (sinm: -sin top half)
                for psum, dest in ((pq0, qT0), (pq1, qT1), (pk, kT)):
                    sw = rope.tile([HD, NT], F32, tag="sw")
                    nc.scalar.copy(sw[0:64, :], psum[64:128, :])
                    nc.scalar.copy(sw[64:128, :], psum[0:64, :])
                    nc.vector.tensor_tensor(sw[:], sw[:], sinm_s[:, t0:t0 + NT], AluOpType.mult)
                    d = dest[:, t0:t0 + NT]
                    nc.vector.tensor_tensor(d, psum[:], cos_s[:, t0:t0 + NT], AluOpType.mult)
                    nc.vector.tensor_tensor(d, d, sw[:], AluOpType.add)
                nc.scalar.copy(vT[:, t0:t0 + NT], pv[:])

        # ---------------- phase 1.5: v = transpose(vT) ----------------
        with tc.tile_pool(name="pst", bufs=2, space="PSUM") as pst:
            for i in range(T // 128):
                tp = pst.tile([128, 128], F32R, tag="tp")
                nc.tensor.matmul(tp[:], vT[:, i * 128:(i + 1) * 128], ident[:],
                                 is_transpose=True)
                nc.vector.tensor_copy(v_sb[:, i, :], tp[:])

    # ---------- attention + out_proj, interleaved per q-tile ----------
    with ExitStack() as p2:
        mpool = p2.enter_context(tc.tile_pool(name="mpool", bufs=1))
        wopool = p2.enter_context(tc.tile_pool(name="wopool", bufs=1))
        ppool = p2.enter_context(tc.tile_pool(name="ppool", bufs=4))
        npool = p2.enter_context(tc.tile_pool(name="npool", bufs=2))
        obpool = p2.enter_context(tc.tile_pool(name="obpool", bufs=2))
        # PSUM budget (8 banks): shared 2-bank slots x3 (scores pairs, bc,
        # out_proj tiles) + ctx 1 + sumexp 1
        pss = p2.enter_context(tc.tile_pool(name="pss", bufs=3, space="PSUM"))
        psc = p2.enter_context(tc.tile_pool(name="psc", bufs=1, space="PSUM"))
        psn = p2.enter_context(tc.tile_pool(name="psn", bufs=1, space="PSUM"))

        masks_s = mpool.tile([128, 4, NT], F32)
        nc.sync.dma_start(masks_s[:], masks_d.rearrange("m p j -> p m j"))
        wo0 = wopool.tile([HD, E], F32R)
        wo1 = wopool.tile([HD, E], F32R)
        nc.sync.dma_start(wo0[:], wo_d[0:HD, :])
        nc.sync.dma_start(wo1[:], wo_d[HD:2 * HD, :])

        for b in range(B):
            base = b * S
            for qt in range(QT_PER_B):
                npairs = 2 * (qt + 1)
                nk = 2 * npairs
                q_sl = slice(base + qt * NT, base + (qt + 1) * NT)
                # ---- attention for both heads on this q tile ----
                for h, (qT_h, ctx_h) in enumerate(((qT0, ctx0), (qT1, ctx1))):
                    ctxp = psc.tile([HD, NT], F32, tag="ctx")
                    sump = psn.tile([1, NT], F32, tag="sum")

                    def emit_ctx_sum(pexp, j):
                        for half in (0, 1):
                            kc = 2 * j + half
                            st, sp_ = (kc == 0), (kc == nk - 1)
                            nc.tensor.matmul(
                                ctxp[:], v_sb[:, b * KB_PER_B + kc, :],
                                pexp[:, half, :], start=st, stop=sp_)
                            nc.tensor.matmul(
                                sump[:], ones_col[:], pexp[:, half, :],
                                start=st, stop=sp_)

                    prev = None
                    for j in range(npairs):
                        sp2 = pss.tile([128, 2, NT], F32, tag="s")
                        for half in (0, 1):
                            kc = 2 * j + half
                            nc.tensor.matmul(
                                sp2[:, half, :],
                                kT[:, base + kc * 128: base + (kc + 1) * 128],
                                qT_h[:, q_sl])
                        pexp = ppool.tile([128, 2, NT], F32R, tag="p")
                        nc.scalar.activation(pexp[:], sp2[:], EXP)
                        if j >= 2 * qt:  # both halves are diagonal chunks
                            mp = slice(0, 2) if j == 2 * qt else slice(2, 4)
                            nc.vector.tensor_tensor(
                                pexp[:], pexp[:], masks_s[:, mp, :],
                                AluOpType.mult)
                        if prev is not None:
                            emit_ctx_sum(*prev)
                        prev = (pexp, j)
                    emit_ctx_sum(*prev)
                    # normalize: ctx_h = ctxp * recip(broadcast(sumexp))
                    sum_sb = npool.tile([1, NT], F32R, tag="ssb")
                    nc.vector.tensor_copy(sum_sb[:], sump[:])
                    bcp = pss.tile([128, 2, NT], F32, tag="s")
                    nc.tensor.matmul(bcp[:, 0, :], ones_row[:], sum_sb[:])
                    bcs = npool.tile([128, NT], F32, tag="bcs")
                    nc.vector.reciprocal_approx_fast(bcs[:], bcp[:, 0, :])
                    nc.vector.tensor_tensor(ctx_h[:, q_sl], ctxp[:], bcs[:],
                                            AluOpType.mult)

                # ---- out_proj for this q tile's 4 token chunks ----
                for tc4 in range(NT // 128):
                    tch = qt * (NT // 128) + tc4
                    tsl = slice(base + tch * 128, base + (tch + 1) * 128)
                    ob = obpool.tile([128, E], F32, tag="ob")
                    for ech in range(E // NT):
                        esl = slice(ech * NT, (ech + 1) * NT)
                        op2 = pss.tile([128, 2, NT], F32, tag="s")
                        op = op2[:, 0, :]
                        nc.tensor.matmul(op, ctx0[:, tsl], wo0[:, esl],
                                         start=True, stop=False)
                        nc.tensor.matmul(op, ctx1[:, tsl], wo1[:, esl],
                                         start=False, stop=True)
                        if ech % 2 == 0:
                            nc.vector.tensor_copy(ob[:, esl], op)
                        else:
                            nc.scalar.copy(ob[:, esl], op)
                    nc.sync.dma_start(out_d[tsl, :], ob[:])


def _build():
    if "nc" in _CACHE:
        return _CACHE["nc"]
    nc = bacc.Bacc("TRN2", target_bir_lowering=False, debug=False,
                   num_devices=NCORES)
    with tile.TileContext(nc) as tc:
        with nc.allow_low_precision(reason="float32r operands for full-rate PE"):
            with ExitStack() as ctx:
                _emit(nc, tc, ctx)
    nc.compile()
    _CACHE["nc"] = nc
    return nc


def _host_consts():
    if "consts" in _CACHE:
        return _CACHE["consts"]
    # RoPE tables, computed in float32 like the reference
    inv_freq = (1.0 / (ROPE_BASE ** (np.arange(0, HD, 2, dtype=np.float32) / HD))
                ).astype(np.float32)
    t = np.arange(S, dtype=np.float32)
    freqs = np.outer(t, inv_freq).astype(np.float32)          # [S, 64]
    emb = np.concatenate([freqs, freqs], axis=-1)             # [S, HD]
    cos_t = np.cos(emb).T.astype(np.float32)                  # [HD, S]
    sin_t = np.sin(emb).T.astype(np.float32)
    sinm_t = np.concatenate([-sin_t[:64], sin_t[64:]], axis=0)
    cos2 = np.ascontiguousarray(np.concatenate([cos_t] * B, axis=1))   # [HD, T]
    sinm2 = np.ascontiguousarray(np.concatenate([sinm_t] * B, axis=1))
    # causal masks for the 4 diagonal 128-chunk offsets within a 512 q-tile
    p = np.arange(128)[:, None]
    j = np.arange(NT)[None, :]
    masks = np.stack([(m * 128 + p <= j) for m in range(4)]).astype(np.float32)
    ident = np.eye(128, dtype=np.float32)
    _CACHE["consts"] = (cos2, sinm2, masks, ident)
    return _CACHE["consts"]


def kernel(x, wq, wk, wv, wo, attn_mask):
    nc = _build()
    cos2, sinm2, masks, ident = _host_consts()
    x = np.asarray(x, dtype=np.float32)
    xT = np.ascontiguousarray(x.reshape(T, E).T)              # [E, T]
    scale = np.float32(1.0 / np.sqrt(HD))
    in_maps = []
    for d in range(NCORES):
        g = d // 2
        in_maps.append({
            "xT": xT,
            "wq": np.ascontiguousarray(wq[:, d * 2 * HD:(d + 1) * 2 * HD]) * scale,
            "wk": np.ascontiguousarray(wk[:, g * HD:(g + 1) * HD]),
            "wv": np.ascontiguousarray(wv[:, g * HD:(g + 1) * HD]),
            "wo": np.ascontiguousarray(wo[d * 2 * HD:(d + 1) * 2 * HD, :]),
            "cos": cos2, "sinm": sinm2, "masks": masks, "ident": ident,
            "onec": np.ones((128, 1), np.float32),
            "oner": np.ones((1, 128), np.float32),
        })
    res = run_bass_kernel_spmd(nc, in_maps, list(range(NCORES)))
    out = res.results[0]["out"].astype(np.float64)
    for d in range(1, NCORES):
        out += res.results[d]["out"]
    return out.astype(np.float32).reshape(B, S, E)


# revision 9
# speedup vs baseline: 1.1165x; 1.1165x over previous
"""GroupedQueryAttention TRN2 Bass kernel, 8-way tensor-parallel over heads.

B=2, S=2048, E=2048, H=16 q-heads, KVH=4 kv-heads, HD=128.
Core d handles q-heads {2d, 2d+1} and kv-head d//2 (redundantly with its pair).

Layout strategy (everything transposed so all matmuls use natural layouts):
  phase 1: qT/kT/vT[hd, tok] = W.T @ xT  (lhsT = W chunks, rhs = xT chunks)
           + RoPE applied on PSUM->SBUF epilogue (1/sqrt(HD) folded into wq)
  phase 1.5: v = transpose(vT) via PE transpose (needed as ctx-matmul lhsT)
  attention (per batch, head, 512-wide q tile): flash-style over PAIRS of
           128-wide kt chunks: scoresT[kt, qt] = kT_chunk.T @ qT_tile -> one
           exp over both chunks [128,1024] (no max subtraction; scores are
           ~N(0,1)) -> causal mask on diagonal chunks -> ctxT[hd, qt] +=
           v_chunk.T @ P; sumexp[1, qt] += ones.T @ P.  PE runs one chunk
           pair ahead of ACT so exp latency is hidden.  Normalize ctxT with
           broadcast (K=1 ones matmul) + reciprocal_approx_fast.
  out_proj (per batch, right after that batch's attention so its output DMA
           overlaps the next batch's attention): out[tok, e] = sum_h
           ctxT_h_chunk.T @ wo_h (partial over this core's 256 head dims;
           host sums the 8 partials).

All matmul operands are float32r (full PE rate at N>=512; ~1e-3 precision).
"""
import sys
sys.path.insert(0, '/opt/trn_rl_repo')

import numpy as np
from contextlib import ExitStack

import concourse.bass as bass
import concourse.bacc as bacc
import concourse.tile as tile
from concourse import mybir
from concourse.bass_utils import run_bass_kernel_spmd
from concourse.alu_op_type import AluOpType

F32 = mybir.dt.float32
F32R = mybir.dt.float32r
EXP = mybir.ActivationFunctionType.Exp

B, S, E = 2, 2048, 2048
H, KVH, HD = 16, 4, 128
T = B * S                  # 4096 flat tokens
NCORES = 8
NT = 512                   # token tile (matmul free dim)
NTT = T // NT              # 8 token tiles
KC = E // 128              # 16 contraction chunks for projections
QT_PER_B = S // NT         # 4 q-tiles per batch
KB_PER_B = S // 128        # 16 kt chunks per batch
ROPE_BASE = 10000.0

_CACHE = {}


def _emit(nc, tc, ctx):
    xT_d = nc.declare_dram_parameter("xT", [E, T], F32R, isOutput=False)
    wq_d = nc.declare_dram_parameter("wq", [E, 2 * HD], F32R, isOutput=False)
    wk_d = nc.declare_dram_parameter("wk", [E, HD], F32R, isOutput=False)
    wv_d = nc.declare_dram_parameter("wv", [E, HD], F32R, isOutput=False)
    wo_d = nc.declare_dram_parameter("wo", [2 * HD, E], F32R, isOutput=False)
    cos_d = nc.declare_dram_parameter("cos", [HD, T], F32, isOutput=False)
    sinm_d = nc.declare_dram_parameter("sinm", [HD, T], F32, isOutput=False)
    masks_d = nc.declare_dram_parameter("masks", [4, 128, NT], F32, isOutput=False)
    ident_d = nc.declare_dram_parameter("ident", [128, 128], F32R, isOutput=False)
    onec_d = nc.declare_dram_parameter("onec", [128, 1], F32R, isOutput=False)
    oner_d = nc.declare_dram_parameter("oner", [1, 128], F32R, isOutput=False)
    out_d = nc.declare_dram_parameter("out", [T, E], F32, isOutput=True)

    persist = ctx.enter_context(tc.tile_pool(name="persist", bufs=1))
    qT0 = persist.tile([HD, T], F32R)
    qT1 = persist.tile([HD, T], F32R)
    kT = persist.tile([HD, T], F32R)
    v_sb = persist.tile([128, T // 128, HD], F32R)   # v natural: [tok%128, blk, hd]
    ctx0 = persist.tile([HD, T], F32R)
    ctx1 = persist.tile([HD, T], F32R)
    ident = persist.tile([128, 128], F32R)
    ones_col = persist.tile([128, 1], F32R)
    ones_row = persist.tile([1, 128], F32R)
    nc.sync.dma_start(ident[:], ident_d[:, :])
    nc.sync.dma_start(ones_col[:], onec_d[:, :])
    nc.sync.dma_start(ones_row[:], oner_d[:, :])

    # ---------------- phase 1: projections + RoPE ----------------
    with ExitStack() as p1x:
        vTp = p1x.enter_context(tc.tile_pool(name="vTp", bufs=1))
        vT = vTp.tile([HD, T], F32R)
        with ExitStack() as p1:
            wpool = p1.enter_context(tc.tile_pool(name="wpool", bufs=1))
            trig = p1.enter_context(tc.tile_pool(name="trig", bufs=1))
            xpool = p1.enter_context(tc.tile_pool(name="xpool", bufs=3))
            rope = p1.enter_context(tc.tile_pool(name="rope", bufs=2))
            ps1 = p1.enter_context(tc.tile_pool(name="ps1", bufs=2, space="PSUM"))

            wq_s = wpool.tile([128, KC, 2 * HD], F32R)
            wk_s = wpool.tile([128, KC, HD], F32R)
            wv_s = wpool.tile([128, KC, HD], F32R)
            cos_s = trig.tile([HD, T], F32)
            sinm_s = trig.tile([HD, T], F32)
            # weight chunk-0 + first x tile first, so the first matmuls
            # start ~15us in instead of waiting for all 12MB of constants
            wqv = wq_d.rearrange("(k p) m -> p k m", p=128)
            wkv = wk_d.rearrange("(k p) m -> p k m", p=128)
            wvv = wv_d.rearrange("(k p) m -> p k m", p=128)
            xT_view = xT_d.rearrange("(k p) t -> p k t", p=128)

            def load_wchunk(kq):
                ks = slice(4 * kq, 4 * kq + 4)
                nc.sync.dma_start(wq_s[:, ks, :], wqv[:, ks, :])
                nc.sync.dma_start(wk_s[:, ks, :], wkv[:, ks, :])
                nc.sync.dma_start(wv_s[:, ks, :], wvv[:, ks, :])

            def load_xk(tt):
                t0 = tt * NT
                xk = []
                for kq in range(4):  # 4 DMAs x 4 chunks of [128, NT]
                    xt = xpool.tile([128, 4, NT], F32R, tag="xk")
                    nc.sync.dma_start(
                        xt[:], xT_view[:, 4 * kq:4 * kq + 4, t0:t0 + NT])
                    xk.append(xt)
                return xk

            load_wchunk(0)
            xk_pre = [load_xk(0), load_xk(1)]
            for kq in range(1, 4):
                load_wchunk(kq)
            nc.sync.dma_start(cos_s[:], cos_d[:, :])
            nc.sync.dma_start(sinm_s[:], sinm_d[:, :])

            for tt in range(NTT):
                t0 = tt * NT
                xk = xk_pre.pop(0)
                if tt + 2 < NTT:
                    xk_pre.append(load_xk(tt + 2))
                pq0 = ps1.tile([HD, NT], F32, tag="pq0")
                pq1 = ps1.tile([HD, NT], F32, tag="pq1")
                pk = ps1.tile([HD, NT], F32, tag="pk")
                pv = ps1.tile([HD, NT], F32, tag="pv")
                for k in range(KC):
                    xck = xk[k // 4][:, k % 4, :]
                    st, sp = (k == 0), (k == KC - 1)
                    nc.tensor.matmul(pq0[:], wq_s[:, k, 0:HD], xck, start=st, stop=sp)
                    nc.tensor.matmul(pq1[:], wq_s[:, k, HD:2 * HD], xck, start=st, stop=sp)
                    nc.tensor.matmul(pk[:], wk_s[:, k, :], xck, start=st, stop=sp)
                    nc.tensor.matmul(pv[:], wv_s[:, k, :], xck, start=st, stop=sp)
                # RoPE: q' = q*cos + swap_halves(q)*sinm  (sinm: -sin top half)
                for psum, dest in ((pq0, qT0), (pq1, qT1), (pk, kT)):
                    sw = rope.tile([HD, NT], F32, tag="sw")
                    nc.scalar.copy(sw[0:64, :], psum[64:128, :])
                    nc.scalar.copy(sw[64:128, :], psum[0:64, :])
                    nc.vector.tensor_tensor(sw[:], sw[:], sinm_s[:, t0:t0 + NT], AluOpType.mult)
                    d = dest[:, t0:t0 + NT]
                    nc.vector.tensor_tensor(d, psum[:], cos_s[:, t0:t0 + NT], AluOpType.mult)
                    nc.vector.tensor_tensor(d, d, sw[:], AluOpType.add)
                nc.scalar.copy(vT[:, t0:t0 + NT], pv[:])

        # ---------------- phase 1.5: v = transpose(vT) ----------------
        with tc.tile_pool(name="pst", bufs=2, space="PSUM") as pst:
            for i in range(T // 128):
                tp = pst.tile([128, 128], F32R, tag="tp")
                nc.tensor.matmul(tp[:], vT[:, i * 128:(i + 1) * 128], ident[:],
                                 is_transpose=True)
                nc.vector.tensor_copy(v_sb[:, i, :], tp[:])

    # ---------- attention + out_proj, interleaved per q-tile ----------
    with ExitStack() as p2:
        mpool = p2.enter_context(tc.tile_pool(name="mpool", bufs=1))
        wopool = p2.enter_context(tc.tile_pool(name="wopool", bufs=1))
        ppool = p2.enter_context(tc.tile_pool(name="ppool", bufs=4))
        npool = p2.enter_context(tc.tile_pool(name="npool", bufs=2))
        obpool = p2.enter_context(tc.tile_pool(name="obpool", bufs=2))
        # PSUM budget (8 banks): scores pairs 2x2 (+bc shared) | ctx 1 |
        # sumexp 1 | out_proj 2
        pss = p2.enter_context(tc.tile_pool(name="pss", bufs=2, space="PSUM"))
        psc = p2.enter_context(tc.tile_pool(name="psc", bufs=1, space="PSUM"))
        psn = p2.enter_context(tc.tile_pool(name="psn", bufs=1, space="PSUM"))
        pso = p2.enter_context(tc.tile_pool(name="pso", bufs=2, space="PSUM"))

        masks_s = mpool.tile([128, 4, NT], F32)
        nc.sync.dma_start(masks_s[:], masks_d.rearrange("m p j -> p m j"))
        wo0 = wopool.tile([HD, E], F32R)
        wo1 = wopool.tile([HD, E], F32R)
        nc.sync.dma_start(wo0[:], wo_d[0:HD, :])
        nc.sync.dma_start(wo1[:], wo_d[HD:2 * HD, :])

        for b in range(B):
            base = b * S
            for qt in range(QT_PER_B):
                npairs = 2 * (qt + 1)
                nk = 2 * npairs
                q_sl = slice(base + qt * NT, base + (qt + 1) * NT)
                # ---- attention for both heads on this q tile ----
                for h, (qT_h, ctx_h) in enumerate(((qT0, ctx0), (qT1, ctx1))):
                    ctxp = psc.tile([HD, NT], F32, tag="ctx")
                    sump = psn.tile([1, NT], F32, tag="sum")

                    def emit_ctx_sum(pexp, j):
                        for half in (0, 1):
                            kc = 2 * j + half
                            st, sp_ = (kc == 0), (kc == nk - 1)
                            nc.tensor.matmul(
                                ctxp[:], v_sb[:, b * KB_PER_B + kc, :],
                                pexp[:, half, :], start=st, stop=sp_)
                            nc.tensor.matmul(
                                sump[:], ones_col[:], pexp[:, half, :],
                                start=st, stop=sp_)

                    prev = None
                    for j in range(npairs):
                        sp2 = pss.tile([128, 2, NT], F32, tag="s")
                        for half in (0, 1):
                            kc = 2 * j + half
                            nc.tensor.matmul(
                                sp2[:, half, :],
                                kT[:, base + kc * 128: base + (kc + 1) * 128],
                                qT_h[:, q_sl])
                        pexp = ppool.tile([128, 2, NT], F32R, tag="p")
                        nc.scalar.activation(pexp[:], sp2[:], EXP)
                        if j >= 2 * qt:  # both halves are diagonal chunks
                            mp = slice(0, 2) if j == 2 * qt else slice(2, 4)
                            nc.vector.tensor_tensor(
                                pexp[:], pexp[:], masks_s[:, mp, :],
                                AluOpType.mult)
                        if prev is not None:
                            emit_ctx_sum(*prev)
                        prev = (pexp, j)
                    emit_ctx_sum(*prev)
                    # drain PSUM into SBUF immediately so the banks free up,
                    # then normalize entirely in SBUF off the PE critical path
                    ctmp = npool.tile([HD, NT], F32, tag="ctmp")
                    nc.vector.tensor_copy(ctmp[:], ctxp[:])
                    sum_sb = npool.tile([1, NT], F32R, tag="ssb")
                    nc.vector.tensor_copy(sum_sb[:], sump[:])
                    bcp = pss.tile([128, 2, NT], F32, tag="s")
                    nc.tensor.matmul(bcp[:, 0, :], ones_row[:], sum_sb[:])
                    bcs = npool.tile([128, NT], F32, tag="bcs")
                    nc.vector.reciprocal_approx_fast(bcs[:], bcp[:, 0, :])
                    nc.vector.tensor_tensor(ctx_h[:, q_sl], ctmp[:], bcs[:],
                                            AluOpType.mult)

                # ---- out_proj for this q tile's 4 token chunks ----
                for tc4 in range(NT // 128):
                    tch = qt * (NT // 128) + tc4
                    tsl = slice(base + tch * 128, base + (tch + 1) * 128)
                    ob = obpool.tile([128, E], F32, tag="ob")
                    for ech in range(E // NT):
                        esl = slice(ech * NT, (ech + 1) * NT)
                        op = pso.tile([128, NT], F32, tag="o")
                        nc.tensor.matmul(op[:], ctx0[:, tsl], wo0[:, esl],
                                         start=True, stop=False)
                        nc.tensor.matmul(op[:], ctx1[:, tsl], wo1[:, esl],
                                         start=False, stop=True)
                        if ech % 2 == 0:
                            nc.vector.tensor_copy(ob[:, esl], op[:])
                        else:
                            nc.scalar.copy(ob[:, esl], op[:])
                    nc.sync.dma_start(out_d[tsl, :], ob[:])


def _build():
    if "nc" in _CACHE:
        return _CACHE["nc"]
    nc = bacc.Bacc("TRN2", target_bir_lowering=False, debug=False,
                   num_devices=NCORES)
    with tile.TileContext(nc) as tc:
        with nc.allow_low_precision(reason="float32r operands for full-rate PE"):
            with ExitStack() as ctx:
                _emit(nc, tc, ctx)
    nc.compile()
    _CACHE["nc"] = nc
    return nc


def _host_consts():
    if "consts" in _CACHE:
        return _CACHE["consts"]
    # RoPE tables, computed in float32 like the reference
    inv_freq = (1.0 / (ROPE_BASE ** (np.arange(0, HD, 2, dtype=np.float32) / HD))
                ).astype(np.float32)
    t = np.arange(S, dtype=np.float32)
    freqs = np.outer(t, inv_freq).astype(np.float32)          # [S, 64]
    emb = np.concatenate([freqs, freqs], axis=-1)             # [S, HD]
    cos_t = np.cos(emb).T.astype(np.float32)                  # [HD, S]
    sin_t = np.sin(emb).T.astype(np.float32)
    sinm_t = np.concatenate([-sin_t[:64], sin_t[64:]], axis=0)
    cos2 = np.ascontiguousarray(np.concatenate([cos_t] * B, axis=1))   # [HD, T]
    sinm2 = np.ascontiguousarray(np.concatenate([sinm_t] * B, axis=1))
    # causal masks for the 4 diagonal 128-chunk offsets within a 512 q-tile
    p = np.arange(128)[:, None]
    j = np.arange(NT)[None, :]
    masks = np.stack([(m * 128 + p <= j) for m in range(4)]).astype(np.float32)
    ident = np.eye(128, dtype=np.float32)
    _CACHE["consts"] = (cos2, sinm2, masks, ident)
    return _CACHE["consts"]


def kernel(x, wq, wk, wv, wo, attn_mask):
    nc = _build()
    cos2, sinm2, masks, ident = _host_consts()
    x = np.asarray(x, dtype=np.float32)
    xT = np.ascontiguousarray(x.reshape(T, E).T)              # [E, T]
    scale = np.float32(1.0 / np.sqrt(HD))
    in_maps = []
    for d in range(NCORES):
        g = d // 2
        in_maps.append({
            "xT": xT,
            "wq": np.ascontiguousarray(wq[:, d * 2 * HD:(d + 1) * 2 * HD]) * scale,
            "wk": np.ascontiguousarray(wk[:, g * HD:(g + 1) * HD]),
            "wv": np.ascontiguousarray(wv[:, g * HD:(g + 1) * HD]),
            "wo": np.ascontiguousarray(wo[d * 2 * HD:(d + 1) * 2 * HD, :]),
            "cos": cos2, "sinm": sinm2, "masks": masks, "ident": ident,
            "onec": np.ones((128, 1), np.float32),
            "oner": np.ones((1, 128), np.float32),
        })
    res = run_bass_kernel_spmd(nc, in_maps, list(range(NCORES)))
    out = res.results[0]["out"].astype(np.float64)
    for d in range(1, NCORES):
        out += res.results[d]["out"]
    return out.astype(np.float32).reshape(B, S, E)
